# revision 3
# baseline (speedup 1.0000x reference)
"""Trainium2 Bass kernel for BasicPGCBlock:
   per-pixel Gaussian smoothing (5x5, sigma = cubic(perspective)) -> dilated 3x3 conv (256->256) + bias + ReLU.

Sharding: data-parallel over batch, 1 image per NeuronCore (8 cores).

Math: the per-pixel 5x5 kernel w(u,v) = exp(-(u^2+v^2)/(2 s^2)) / Z factors through
t = exp(-1/(2 s^2)):  w(u,v) = t^(u^2+v^2) / Z, and u^2+v^2 in {0,1,2,4,5,8}.
So smoothed = sum_m c_m * S_m with c_m = t^m / Z (host-computed per-pixel planes,
replicated across partitions) and S_m = fixed 0/1 stencil sums of x built from 11
shifted adds (separable structure). The dilated conv is 9 taps x (2x2) 128-channel
matmul tiles accumulated in PSUM, with bias+ReLU fused into the ScalarE evacuation.

Layout: channels on partitions (2 tiles of 128 fused as an extra free dim), pixels
on the free dimension, all smoothing in bf16 (DVE 2x mode), conv in bf16 with f32
PSUM accumulation.
"""

import sys

sys.path.insert(0, "/opt/trn_rl_repo")

import numpy as np
import ml_dtypes

BF16 = ml_dtypes.bfloat16

B, C, H, W = 8, 256, 96, 96
HP, WP = H + 4, W + 4          # zero-padded by 2 on each side
SLAB = 16                      # interior rows per smoothing slab
NSLAB = H // SLAB
CHUNK = 4                      # conv output rows per matmul (N = 4*96 = 384 <= 512)
NCHUNK = SLAB // CHUNK
OFFS = (-2, 0, 2)              # dilated conv offsets
MS = (0, 1, 2, 4, 5, 8)        # exponents of t present in the 5x5 kernel

_cache = {}


def _build(repeats=1):
    import concourse.mybir as mybir
    from concourse import bacc
    from concourse.tile import TileContext

    dt = mybir.dt
    nc = bacc.Bacc("TRN2", target_bir_lowering=False, debug=False)

    xp = nc.dram_tensor("xp", (128, 2, HP, WP), dt.bfloat16, kind="ExternalInput").ap()
    cpl = nc.dram_tensor("cpl", (128, 6, H, W), dt.bfloat16, kind="ExternalInput").ap()
    wts = nc.dram_tensor("wts", (2, 128, 9 * 2 * 128), dt.bfloat16, kind="ExternalInput").ap()
    bias = nc.dram_tensor("bias", (128, 2), dt.float32, kind="ExternalInput").ap()
    y = nc.dram_tensor("y", (2, 128, H, W), dt.float32, kind="ExternalOutput").ap()

    with TileContext(nc) as tc:
        with (
            tc.tile_pool(name="const", bufs=1) as constp,
            tc.tile_pool(name="smpool", bufs=1) as smpool,
            tc.tile_pool(name="io", bufs=2) as iop,
            tc.tile_pool(name="tmp", bufs=1) as tmp,
            tc.tile_pool(name="outp", bufs=6) as outp,
            tc.tile_pool(name="psum", bufs=8, space="PSUM") as psp,
        ):
            w_sb = constp.tile([128, 2, 9 * 2 * 128], dt.bfloat16)
            nc.sync.dma_start(out=w_sb[:, 0], in_=wts[0])
            nc.sync.dma_start(out=w_sb[:, 1], in_=wts[1])
            b_sb = constp.tile([128, 2], dt.float32)
            nc.sync.dma_start(out=b_sb, in_=bias)

            sm = smpool.tile([128, 2, HP, WP], dt.bfloat16)
            nc.vector.memset(sm[:], 0.0)

            def smooth(s):
                r0 = SLAB * s
                cp = iop.tile([128, 6, SLAB, W], dt.bfloat16, name="cp")
                nc.sync.dma_start(out=cp, in_=cpl[:, :, r0 : r0 + SLAB, :])
                xs = iop.tile([128, 2, SLAB + 4, WP], dt.bfloat16, name="xs")
                nc.sync.dma_start(out=xs, in_=xp[:, :, r0 : r0 + SLAB + 4, :])

                def cpm(m):
                    i = MS.index(m)
                    return cp[:, i : i + 1].to_broadcast([128, 2, SLAB, W])

                P0 = xs[:, :, :, 2 : W + 2]
                P1 = tmp.tile([128, 2, SLAB + 4, W], dt.bfloat16, name="P1")
                nc.vector.tensor_add(P1, xs[:, :, :, 1 : W + 1], xs[:, :, :, 3 : W + 3])
                P2 = tmp.tile([128, 2, SLAB + 4, W], dt.bfloat16, name="P2")
                nc.vector.tensor_add(P2, xs[:, :, :, 0:W], xs[:, :, :, 4 : W + 4])

                ctr = lambda P: P[:, :, 2 : SLAB + 2]
                u1 = lambda P: P[:, :, 1 : SLAB + 1]
                d1 = lambda P: P[:, :, 3 : SLAB + 3]
                u2 = lambda P: P[:, :, 0:SLAB]
                d2 = lambda P: P[:, :, 4 : SLAB + 4]

                acc = tmp.tile([128, 2, SLAB, W], dt.bfloat16, name="acc")
                nc.vector.tensor_mul(acc, ctr(P0), cpm(0))

                sm_out = sm[:, :, 2 + r0 : 2 + r0 + SLAB, 2 : W + 2]

                def term(S, m, last=False):
                    t = tmp.tile([128, 2, SLAB, W], dt.bfloat16, name="t")
                    nc.vector.tensor_mul(t, S, cpm(m))
                    nc.vector.tensor_add(sm_out if last else acc, acc, t)

                # m=1: S1 = (P0[h-1]+P0[h+1]) + P1[h]
                Qa = tmp.tile([128, 2, SLAB, W], dt.bfloat16, name="Qa")
                nc.vector.tensor_add(Qa, u1(P0), d1(P0))
                S = tmp.tile([128, 2, SLAB, W], dt.bfloat16, name="S")
                nc.vector.tensor_add(S, Qa, ctr(P1))
                term(S, 1)
                # m=2: S2 = P1[h-1]+P1[h+1]
                S = tmp.tile([128, 2, SLAB, W], dt.bfloat16, name="S")
                nc.vector.tensor_add(S, u1(P1), d1(P1))
                term(S, 2)
                # m=4: S4 = (P0[h-2]+P0[h+2]) + P2[h]
                Qa = tmp.tile([128, 2, SLAB, W], dt.bfloat16, name="Qa")
                nc.vector.tensor_add(Qa, u2(P0), d2(P0))
                S = tmp.tile([128, 2, SLAB, W], dt.bfloat16, name="S")
                nc.vector.tensor_add(S, Qa, ctr(P2))
                term(S, 4)
                # m=5: S5 = (P1[h-2]+P1[h+2]) + (P2[h-1]+P2[h+1])
                Qa = tmp.tile([128, 2, SLAB, W], dt.bfloat16, name="Qa")
                nc.vector.tensor_add(Qa, u2(P1), d2(P1))
                Qb = tmp.tile([128, 2, SLAB, W], dt.bfloat16, name="Qb")
                nc.vector.tensor_add(Qb, u1(P2), d1(P2))
                S = tmp.tile([128, 2, SLAB, W], dt.bfloat16, name="S")
                nc.vector.tensor_add(S, Qa, Qb)
                term(S, 5)
                # m=8: S8 = P2[h-2]+P2[h+2]
                S = tmp.tile([128, 2, SLAB, W], dt.bfloat16, name="S")
                nc.vector.tensor_add(S, u2(P2), d2(P2))
                term(S, 8, last=True)

            def conv_chunks(s, chunks):
                r0 = SLAB * s
                for oi in range(2):
                    pcs = {
                        k: psp.tile([128, CHUNK, W], dt.float32, name="pc")
                        for k in chunks
                    }
                    for idx in range(18):
                        ki, q = idx // 9, idx % 9
                        dh, dw = OFFS[q // 3], OFFS[q % 3]
                        lhsT = w_sb[:, ki, (q * 2 + oi) * 128 : (q * 2 + oi + 1) * 128]
                        for k in chunks:
                            rr = r0 + CHUNK * k
                            rhs = sm[
                                :, ki, 2 + rr + dh : 2 + rr + CHUNK + dh, 2 + dw : 2 + dw + W
                            ]
                            nc.tensor.matmul(
                                pcs[k], lhsT, rhs, start=(idx == 0), stop=(idx == 17)
                            )
                    for k in chunks:
                        rr = r0 + CHUNK * k
                        ob = outp.tile([128, CHUNK, W], dt.float32, name="ob")
                        nc.scalar.activation(
                            ob,
                            pcs[k],
                            mybir.ActivationFunctionType.Relu,
                            bias=b_sb[:, oi : oi + 1],
                            scale=1.0,
                        )
                        nc.sync.dma_start(out=y[oi, :, rr : rr + CHUNK, :], in_=ob)

            for _ in range(repeats):
                early = tuple(range(NCHUNK - 1))  # chunks safe once own slab smoothed
                smooth(0)
                for s in range(1, NSLAB):
                    conv_chunks(s - 1, early)
                    smooth(s)
                    conv_chunks(s - 1, (NCHUNK - 1,))
                conv_chunks(NSLAB - 1, tuple(range(NCHUNK)))

    nc.compile()
    return nc


def _prep(inputs):
    x = np.asarray(inputs["x"], np.float32)
    pm = np.asarray(inputs["perspective_map"], np.float32)
    co = np.asarray(inputs["sigma_coeffs"], np.float32)
    Wc = np.asarray(inputs["conv_w"], np.float32)
    bb = np.asarray(inputs["conv_b"], np.float32)

    # per-pixel coefficient planes (host): c_m = t^m / Z, replicated over partitions
    p = pm[:, 0]  # [B,H,W]
    sigma = co[0] * p**3 + co[1] * p**2 + co[2] * p + co[3]
    sigma = np.maximum(sigma, 0.5)
    t = np.exp(-1.0 / (2.0 * sigma * sigma))
    Z = 1 + 4 * t + 4 * t**2 + 4 * t**4 + 8 * t**5 + 4 * t**8
    cm = np.stack([(t**m) / Z for m in MS], axis=1).astype(BF16)  # [B,6,H,W]
    cpl = np.ascontiguousarray(
        np.broadcast_to(cm[:, None], (B, 128, 6, H, W))
    )  # [B,128,6,H,W]

    # zero-padded bf16 input: [B, 128(part), 2(ct), HP, WP]
    xpad = np.zeros((B, 128, 2, HP, WP), BF16)
    xpad[:, :, :, 2 : H + 2, 2 : W + 2] = (
        x.astype(BF16).reshape(B, 2, 128, H, W).transpose(0, 2, 1, 3, 4)
    )

    # conv weights: lhsT layout [ki, 128(i), q, oi, 128(o)]
    Wt = Wc.transpose(1, 0, 2, 3).astype(BF16)  # [I, O, kh, kw]
    wts = np.empty((2, 128, 9, 2, 128), BF16)
    for ki in range(2):
        for q in range(9):
            kh, kw = q // 3, q % 3
            for oi in range(2):
                wts[ki, :, q, oi, :] = Wt[
                    ki * 128 : (ki + 1) * 128, oi * 128 : (oi + 1) * 128, kh, kw
                ]
    wts = wts.reshape(2, 128, 9 * 2 * 128)
    bias_h = np.ascontiguousarray(bb.reshape(2, 128).T.astype(np.float32))  # [128, 2]

    return [
        {"xp": xpad[b], "cpl": cpl[b], "wts": wts, "bias": bias_h} for b in range(B)
    ]


def _get_nc(repeats=1):
    key = ("nc", repeats)
    if key not in _cache:
        _cache[key] = _build(repeats)
    return _cache[key]


def run(inputs, trace=False, **kw):
    from concourse.bass_utils import run_bass_kernel_spmd

    nc = _get_nc()
    in_maps = _prep(inputs)
    res = run_bass_kernel_spmd(nc, in_maps, core_ids=list(range(B)), trace=trace, **kw)
    out = np.stack(
        [r["y"].reshape(2, 128, H, W).reshape(C, H, W) for r in res.results]
    ).astype(np.float32)
    return out, res


def kernel(**inputs):
    out, _ = run(inputs)
    return out


# revision 38
# speedup vs baseline: 1.4897x; 1.4897x over previous
"""Trainium2 Bass kernel for BasicPGCBlock:
   per-pixel Gaussian smoothing (5x5, sigma = cubic(perspective)) -> dilated 3x3 conv (256->256) + bias + ReLU.

Sharding: data-parallel over batch, 1 image per NeuronCore (8 cores).

Math: the per-pixel 5x5 kernel w(u,v) = exp(-(u^2+v^2)/(2 s^2)) / Z factors through
t = exp(-1/(2 s^2)):  w(u,v) = t^(u^2+v^2) / Z, and u^2+v^2 in {0,1,2,4,5,8}.
So smoothed = sum_m c_m * S_m with c_m = t^m / Z (host-computed per-pixel planes,
replicated across partitions) and S_m = fixed 0/1 stencil sums of x built from
shifted adds (separable structure). S5 is accumulated on TensorE via identity
matmuls to offload the busiest engine (VectorE). The dilated conv is 9 taps x
(2x2) 128-channel matmul tiles accumulated in PSUM, with bias+ReLU fused into the
ScalarE evacuation.

Layout: channels on partitions (2 tiles of 128 fused as an extra free dim), pixels
on the free dimension, all smoothing in bf16 (DVE 2x mode), conv in bf16 with f32
PSUM accumulation. The last row-slabs are small so only a sliver of conv work
trails the final smoothing op.
"""

import sys

sys.path.insert(0, "/opt/trn_rl_repo")

import numpy as np
import ml_dtypes

BF16 = ml_dtypes.bfloat16

B, C, H, W = 8, 256, 96, 96
HP, WP = H + 4, W + 4          # zero-padded by 2 on each side
SLABS = ((0, 16), (16, 16), (32, 16), (48, 16), (64, 16), (80, 16))
CHUNK = 4                      # conv output rows per matmul (N = 4*96 = 384 <= 512)
OFFS = (-2, 0, 2)              # dilated conv offsets
MS = (0, 1, 2, 4, 5, 8)        # exponents of t present in the 5x5 kernel
PE_S5 = True                   # build S5 = P1[h+-2] + P2[h+-1] sums on TensorE
PE_S28 = False                 # also build S2/S8 row sums on TensorE

_cache = {}


def _build(repeats=1, loop=None, pe_s5=None, pe_s28=None):
    pe_s5 = PE_S5 if pe_s5 is None else pe_s5
    pe_s28 = PE_S28 if pe_s28 is None else pe_s28
    import concourse.mybir as mybir
    from concourse import bacc
    from concourse.tile import TileContext

    dt = mybir.dt
    nc = bacc.Bacc("TRN2", target_bir_lowering=False, debug=False)

    xp = nc.dram_tensor("xp", (128, 2, HP, WP), dt.bfloat16, kind="ExternalInput").ap()
    cpl = nc.dram_tensor("cpl", (128, 6, H, W), dt.bfloat16, kind="ExternalInput").ap()
    wts = nc.dram_tensor("wts", (2, 128, 9 * 2 * 128), dt.bfloat16, kind="ExternalInput").ap()
    bias = nc.dram_tensor("bias", (128, 2), dt.float32, kind="ExternalInput").ap()
    ident = nc.dram_tensor("ident", (128, 128), dt.bfloat16, kind="ExternalInput").ap()
    y = nc.dram_tensor("y", (2, 128, H, W), dt.float32, kind="ExternalOutput").ap()

    with TileContext(nc) as tc:
        with (
            tc.tile_pool(name="const", bufs=1) as constp,
            tc.tile_pool(name="smpool", bufs=1) as smpool,
            tc.tile_pool(name="io", bufs=2) as iop,
            tc.tile_pool(name="tmp", bufs=1) as tmp,
            tc.tile_pool(name="outp", bufs=6) as outp,
            tc.tile_pool(name="psum", bufs=8, space="PSUM") as psp,
        ):
            id_sb = constp.tile([128, 128], dt.bfloat16)
            nc.sync.dma_start(out=id_sb, in_=ident)
            w_sb = constp.tile([128, 2, 9 * 2 * 128], dt.bfloat16)
            b_sb = constp.tile([128, 2], dt.float32)

            def load_consts():
                # emitted after the first slab's input DMAs: conv weights are not
                # needed until well into the first slab, keep them off the
                # startup critical path
                nc.sync.dma_start(out=w_sb[:, 0], in_=wts[0])
                nc.sync.dma_start(out=w_sb[:, 1], in_=wts[1])
                nc.sync.dma_start(out=b_sb, in_=bias)

            sm = smpool.tile([128, 2, HP, WP], dt.bfloat16)
            # zero only the 2-wide pad ring; the interior is fully rewritten
            nc.vector.memset(sm[:, :, 0:2, :], 0.0)
            nc.vector.memset(sm[:, :, HP - 2 : HP, :], 0.0)
            nc.vector.memset(sm[:, :, 2 : HP - 2, 0:2], 0.0)
            nc.vector.memset(sm[:, :, 2 : HP - 2, WP - 2 : WP], 0.0)

            def smooth(r0, nr, flush_fn=None):
                use_pe_s5 = pe_s5 and nr >= 16
                xs = iop.tile([128, 2, nr + 4, WP], dt.bfloat16, name="xs")
                nc.sync.dma_start(out=xs, in_=xp[:, :, r0 : r0 + nr + 4, :])
                cp = iop.tile([128, 6, nr, W], dt.bfloat16, name="cp")
                nc.sync.dma_start(out=cp, in_=cpl[:, :, r0 : r0 + nr, :])

                def cpm(m):
                    i = MS.index(m)
                    return cp[:, i : i + 1].to_broadcast([128, 2, nr, W])

                P0 = xs[:, :, :, 2 : W + 2]
                P1 = tmp.tile([128, 2, nr + 4, W], dt.bfloat16, name="P1", bufs=2)
                nc.vector.tensor_add(P1, xs[:, :, :, 1 : W + 1], xs[:, :, :, 3 : W + 3])
                P2 = tmp.tile([128, 2, nr + 4, W], dt.bfloat16, name="P2", bufs=2)
                nc.vector.tensor_add(P2, xs[:, :, :, 0:W], xs[:, :, :, 4 : W + 4])

                ctr = lambda P: P[:, :, 2 : nr + 2]
                u1 = lambda P: P[:, :, 1 : nr + 1]
                d1 = lambda P: P[:, :, 3 : nr + 3]
                u2 = lambda P: P[:, :, 0:nr]
                d2 = lambda P: P[:, :, 4 : nr + 4]

                if use_pe_s5:
                    # S5 = (P1[h-2]+P1[h+2]) + (P2[h-1]+P2[h+1]) accumulated on
                    # TensorE via identity matmuls, evacuated to bf16 by ScalarE.
                    S5 = tmp.tile([128, 2, nr, W], dt.bfloat16, name="S5", bufs=2)
                    for ct in range(2):
                        for rk in range(nr // CHUNK):
                            rs = CHUNK * rk
                            pc5 = psp.tile(
                                [128, CHUNK, W], dt.float32, name="pc5", bufs=2
                            )
                            for j, Pv in enumerate((u2(P1), d2(P1), u1(P2), d1(P2))):
                                nc.tensor.matmul(
                                    pc5,
                                    id_sb,
                                    Pv[:, ct, rs : rs + CHUNK, :],
                                    start=(j == 0),
                                    stop=(j == 3),
                                )
                            nc.scalar.activation(
                                S5[:, ct, rs : rs + CHUNK, :],
                                pc5,
                                mybir.ActivationFunctionType.Copy,
                            )


                if flush_fn is not None:
                    # last slab: build all stencil sums as tiles, then apply in
                    # two row-halves with a conv flush in between so most of the
                    # trailing conv work starts before smoothing finishes.
                    S1 = tmp.tile([128, 2, nr, W], dt.bfloat16, name="S1")
                    nc.vector.tensor_add(S1, u1(P0), d1(P0))
                    nc.vector.tensor_add(S1, S1, ctr(P1))
                    S2 = tmp.tile([128, 2, nr, W], dt.bfloat16, name="S2")
                    nc.vector.tensor_add(S2, u1(P1), d1(P1))
                    S4 = tmp.tile([128, 2, nr, W], dt.bfloat16, name="S4")
                    nc.vector.tensor_add(S4, u2(P0), d2(P0))
                    nc.vector.tensor_add(S4, S4, ctr(P2))
                    if not use_pe_s5:
                        S5 = tmp.tile([128, 2, nr, W], dt.bfloat16, name="S5x")
                        nc.vector.tensor_add(S5, u2(P1), d2(P1))
                        Qb = tmp.tile([128, 2, nr, W], dt.bfloat16, name="Qb")
                        nc.vector.tensor_add(Qb, u1(P2), d1(P2))
                        nc.vector.tensor_add(S5, S5, Qb)
                    S8 = tmp.tile([128, 2, nr, W], dt.bfloat16, name="S8")
                    nc.vector.tensor_add(S8, u2(P2), d2(P2))
                    hn = nr // 2
                    for h0 in (0, hn):
                        if h0:
                            flush_fn(r0 + h0)

                        def cpmh(m):
                            i = MS.index(m)
                            return cp[:, i : i + 1, h0 : h0 + hn, :].to_broadcast(
                                [128, 2, hn, W]
                            )

                        acc = tmp.tile([128, 2, hn, W], dt.bfloat16, name="acc")
                        nc.vector.tensor_mul(
                            acc, ctr(P0)[:, :, h0 : h0 + hn], cpmh(0)
                        )
                        smo = sm[:, :, 2 + r0 + h0 : 2 + r0 + h0 + hn, 2 : W + 2]
                        for Sx, m in ((S1, 1), (S2, 2), (S4, 4), (S5, 5), (S8, 8)):
                            t = tmp.tile([128, 2, hn, W], dt.bfloat16, name="t")
                            nc.vector.tensor_mul(t, Sx[:, :, h0 : h0 + hn], cpmh(m))
                            nc.vector.tensor_add(smo if m == 8 else acc, acc, t)
                    return

                acc = tmp.tile([128, 2, nr, W], dt.bfloat16, name="acc")
                nc.vector.tensor_mul(acc, ctr(P0), cpm(0))

                sm_out = sm[:, :, 2 + r0 : 2 + r0 + nr, 2 : W + 2]

                def term(S, m, last=False):
                    t = tmp.tile([128, 2, nr, W], dt.bfloat16, name="t")
                    nc.vector.tensor_mul(t, S, cpm(m))
                    nc.vector.tensor_add(sm_out if last else acc, acc, t)

                # m=1: S1 = (P0[h-1]+P0[h+1]) + P1[h]
                Qa = tmp.tile([128, 2, nr, W], dt.bfloat16, name="Qa")
                nc.vector.tensor_add(Qa, u1(P0), d1(P0))
                S = tmp.tile([128, 2, nr, W], dt.bfloat16, name="S")
                nc.vector.tensor_add(S, Qa, ctr(P1))
                term(S, 1)
                # m=2: S2 = P1[h-1]+P1[h+1]
                S = tmp.tile([128, 2, nr, W], dt.bfloat16, name="S")
                nc.vector.tensor_add(S, u1(P1), d1(P1))
                term(S, 2)
                # m=4: S4 = (P0[h-2]+P0[h+2]) + P2[h]
                Qa = tmp.tile([128, 2, nr, W], dt.bfloat16, name="Qa")
                nc.vector.tensor_add(Qa, u2(P0), d2(P0))
                S = tmp.tile([128, 2, nr, W], dt.bfloat16, name="S")
                nc.vector.tensor_add(S, Qa, ctr(P2))
                term(S, 4)
                # m=5
                if use_pe_s5:
                    term(S5, 5)
                else:
                    Qa = tmp.tile([128, 2, nr, W], dt.bfloat16, name="Qa")
                    nc.vector.tensor_add(Qa, u2(P1), d2(P1))
                    Qb = tmp.tile([128, 2, nr, W], dt.bfloat16, name="Qb")
                    nc.vector.tensor_add(Qb, u1(P2), d1(P2))
                    S = tmp.tile([128, 2, nr, W], dt.bfloat16, name="S")
                    nc.vector.tensor_add(S, Qa, Qb)
                    term(S, 5)
                # m=8: S8 = P2[h-2]+P2[h+2]
                S = tmp.tile([128, 2, nr, W], dt.bfloat16, name="S")
                nc.vector.tensor_add(S, u2(P2), d2(P2))
                term(S, 8, last=True)

            def conv_group(rrs):
                # rrs: output-row starts whose sm dependencies are met; one
                # LDWEIGHTS serves len(rrs) matmuls.
                for oi in range(2):
                    pcs = [
                        psp.tile([128, CHUNK, W], dt.float32, name="pc", bufs=6)
                        for _ in rrs
                    ]
                    for idx in range(18):
                        ki, q = idx // 9, idx % 9
                        dh, dw = OFFS[q // 3], OFFS[q % 3]
                        lhsT = w_sb[:, ki, (q * 2 + oi) * 128 : (q * 2 + oi + 1) * 128]
                        for j, rr in enumerate(rrs):
                            rhs = sm[
                                :, ki, 2 + rr + dh : 2 + rr + CHUNK + dh, 2 + dw : 2 + dw + W
                            ]
                            nc.tensor.matmul(
                                pcs[j], lhsT, rhs, start=(idx == 0), stop=(idx == 17)
                            )
                    for j, rr in enumerate(rrs):
                        ob = outp.tile([128, CHUNK, W], dt.float32, name="ob")
                        nc.scalar.activation(
                            ob,
                            pcs[j],
                            mybir.ActivationFunctionType.Relu,
                            bias=b_sb[:, oi : oi + 1],
                            scale=1.0,
                        )
                        nc.sync.dma_start(out=y[oi, :, rr : rr + CHUNK, :], in_=ob)

            def body():
                # conv rows rr..rr+3 read smp rows rr..rr+7 (interior rr-2..rr+5):
                # emit each chunk as soon as smoothing covers row rr+5.
                pending = list(range(0, H, CHUNK))
                def flush(upto):
                    ready = [rr for rr in pending if rr + 6 <= upto or upto >= H]
                    for rr in ready:
                        pending.remove(rr)
                    if ready:
                        conv_group(ready)

                for si, (r0, nr) in enumerate(SLABS):
                    # last-slab split apply measured no better on HW and the
                    # model agrees post-double-buffering: the tail is PE-queue
                    # bound, not dependency bound. Keep single-pass emission.
                    smooth(r0, nr, flush_fn=None)
                    if si == 0:
                        load_consts()
                    flush(r0 + nr)
                assert not pending

            if loop is not None:
                with tc.For_i(0, loop, 1):
                    body()
            else:
                for _ in range(repeats):
                    body()

    nc.compile()
    return nc


def _prep(inputs):
    x = np.asarray(inputs["x"], np.float32)
    pm = np.asarray(inputs["perspective_map"], np.float32)
    co = np.asarray(inputs["sigma_coeffs"], np.float32)
    Wc = np.asarray(inputs["conv_w"], np.float32)
    bb = np.asarray(inputs["conv_b"], np.float32)

    # per-pixel coefficient planes (host): c_m = t^m / Z, replicated over partitions
    p = pm[:, 0]  # [B,H,W]
    sigma = co[0] * p**3 + co[1] * p**2 + co[2] * p + co[3]
    sigma = np.maximum(sigma, 0.5)
    t = np.exp(-1.0 / (2.0 * sigma * sigma))
    Z = 1 + 4 * t + 4 * t**2 + 4 * t**4 + 8 * t**5 + 4 * t**8
    cm = np.stack([(t**m) / Z for m in MS], axis=1).astype(BF16)  # [B,6,H,W]
    cpl = np.ascontiguousarray(np.broadcast_to(cm[:, None], (B, 128, 6, H, W)))

    # zero-padded bf16 input: [B, 128(part), 2(ct), HP, WP]
    xpad = np.zeros((B, 128, 2, HP, WP), BF16)
    xpad[:, :, :, 2 : H + 2, 2 : W + 2] = (
        x.astype(BF16).reshape(B, 2, 128, H, W).transpose(0, 2, 1, 3, 4)
    )

    # conv weights: lhsT layout [ki, 128(i), q, oi, 128(o)]
    Wt = Wc.transpose(1, 0, 2, 3).astype(BF16)  # [I, O, kh, kw]
    wts = np.empty((2, 128, 9, 2, 128), BF16)
    for ki in range(2):
        for q in range(9):
            kh, kw = q // 3, q % 3
            for oi in range(2):
                wts[ki, :, q, oi, :] = Wt[
                    ki * 128 : (ki + 1) * 128, oi * 128 : (oi + 1) * 128, kh, kw
                ]
    wts = wts.reshape(2, 128, 9 * 2 * 128)
    bias_h = np.ascontiguousarray(bb.reshape(2, 128).T.astype(np.float32))  # [128, 2]
    ident = np.eye(128, dtype=BF16)

    return [
        {"xp": xpad[b], "cpl": cpl[b], "wts": wts, "bias": bias_h, "ident": ident}
        for b in range(B)
    ]


def _get_nc(repeats=1, loop=None, pe_s5=None, pe_s28=None):
    key = ("nc", repeats, loop, pe_s5, pe_s28)
    if key not in _cache:
        _cache[key] = _build(repeats, loop, pe_s5, pe_s28)
    return _cache[key]


def run(inputs, trace=False, **kw):
    from concourse.bass_utils import run_bass_kernel_spmd

    nc = _get_nc()
    in_maps = _prep(inputs)
    res = run_bass_kernel_spmd(nc, in_maps, core_ids=list(range(B)), trace=trace, **kw)
    out = np.stack([r["y"].reshape(C, H, W) for r in res.results]).astype(np.float32)
    return out, res


def kernel(**inputs):
    out, _ = run(inputs)
    return out
